# revision 1
# baseline (speedup 1.0000x reference)
"""Normalized-adjacency kernel (EstimateAdj.normalize, symmetric=False) for TRN2.

out = mx * r_inv[:, None] * r_inv[None, :]   where mx = adj + I,
r_inv = rowsum(mx) ** -0.5.

Strategy (8 NeuronCores, row-sharded, raw Bass with explicit semaphores):
  - host: add 1.0 to the diagonal (O(n)), split rows into 8 shards
  - device, per core: work items are HALF-tiles [128 x n/2]
    (tile t = shard rows [t*128:(t+1)*128], halves h split the columns):
      pass 1: stream the first 11 halves through 5 SBUF slots, keep the last
              5 halves resident.  Rowsums run on the SCALAR engine
              (activation Copy with accum_out), so the DVE stays free and the
              loads, not the reduces, pace the pass.
      r_inv = 1/sqrt(rowsum) (ACT sqrt + DVE reciprocal); PE transposes
      r_inv via an identity matmul so the DRAM write of the local r_inv is
      8 contiguous 512B descriptors instead of 128 scattered 32B ones.
      AllGather local r_inv (DRAM) -> full n vector; while it is in flight
      the 5 stream slots prefetch the first 5 pass-2 halves (~10 MiB).
      pass 2: fused in-place DVE scalar_tensor_tensor per half:
              half = (half * r_inv_row_scalar) * colscale[:, h-slice]; store.
              Prefetched stream halves are processed FIRST so their stores
              complete early and un-gate the remaining reloads (the reload
              chain is bandwidth-bound, not latency-bound).
  - engines: gpsimd/Pool = loads + allgather; SP/sync = stores + small DMAs;
    DVE = fused scales; ACT = rowsums + sqrt; PE = r_inv transpose.
  - host: concatenate the 8 output shards
"""

from contextlib import ExitStack

import numpy as np

import concourse.bass as bass
import concourse.mybir as mybir
from concourse.bass_utils import run_bass_kernel_spmd

N = 8192
NCORES = 8
SHARD = N // NCORES  # 1024
P = 128
T = SHARD // P  # 8 tiles per core
H = 2  # column halves per tile

F32 = mybir.dt.float32
NSTREAM = 6  # streaming half-tile slots
NCACHE = 4  # pass-1-resident half-tile slots


def build_kernel(n=N, ncores=NCORES):
    shard = n // ncores
    tt = shard // P
    w = n // H  # half width
    items = [(t, h) for t in range(tt) for h in range(H)]  # load order
    ni = len(items)

    ncache = min(NCACHE, max(ni - NSTREAM, 0))
    nstream = min(NSTREAM, ni - ncache)
    stream_items = list(range(ni - ncache))  # indices into `items`
    cached_items = list(range(ni - ncache, ni))

    def slot_of(i):
        if i >= ni - ncache:
            return nstream + (i - (ni - ncache))
        return i % nstream

    # pass-2 order: prefetched stream halves first (their stores un-gate the
    # reloads), then cached halves, then the reloaded stream halves.
    order = (
        stream_items[:nstream] + cached_items + stream_items[nstream:]
    )

    # per-slot cumulative load-completion values (s_in[slot])
    nslots = nstream + ncache
    in_count = [0] * nslots
    in_val1 = [0] * ni
    for i in range(ni):
        in_count[slot_of(i)] += 16
        in_val1[i] = in_count[slot_of(i)]
    in_val2 = {}
    for i in stream_items:
        in_count[slot_of(i)] += 16
        in_val2[i] = in_count[slot_of(i)]

    # per-stream-slot cumulative store-completion values (s_souts[slot])
    souts_count = [0] * max(nstream, 1)
    souts_val = {}
    for i in stream_items:
        souts_count[slot_of(i)] += 16
        souts_val[i] = souts_count[slot_of(i)]

    # rowsum -> r_inv -> transpose -> DRAM chain is pipelined in two groups
    # (all-but-last tile early, last tile late) so most of it hides under the
    # tail of pass 1
    groups = [(0, tt - 1), (tt - 1, tt)] if tt >= 2 else [(0, tt)]
    ng = len(groups)

    nc = bass.Bass(num_devices=ncores)
    mx = nc.dram_tensor("mx", [shard, n], F32, kind="ExternalInput")
    eye = nc.dram_tensor("eye", [P, P], F32, kind="ExternalInput")
    out = nc.dram_tensor("out", [shard, n], F32, kind="ExternalOutput")
    cc_in = nc.dram_tensor("cc_in", [shard], F32)
    cc_out = nc.dram_tensor("cc_out", [n], F32, addr_space="Shared")

    # blocked tiling: tile t, partition p, half h -> shard row t*128 + p
    mx_v = mx.rearrange("(t p) (h w) -> t p h w", p=P, h=H)
    out_v = out.rearrange("(t p) (h w) -> t p h w", p=P, h=H)

    with ExitStack() as ctx:
        slots = [
            ctx.enter_context(nc.sbuf_tensor(f"tile{i}", [P, w], F32))
            for i in range(nslots)
        ]
        colscale = ctx.enter_context(nc.sbuf_tensor("colscale", [P, n], F32))
        eye_sb = ctx.enter_context(nc.sbuf_tensor("eye_sb", [P, P], F32))
        ps = ctx.enter_context(nc.sbuf_tensor("ps", [P, ni], F32))
        rs = ctx.enter_context(nc.sbuf_tensor("rs", [P, tt], F32))
        rinv = ctx.enter_context(nc.sbuf_tensor("rinv", [P, tt], F32))
        ptc = [
            ctx.enter_context(nc.sbuf_tensor(f"ptc{g}", [b - a, P], F32))
            for g, (a, b) in enumerate(groups)
        ]
        pt = [
            ctx.enter_context(nc.psum_tensor(f"pt{g}", [b - a, P], F32))
            for g, (a, b) in enumerate(groups)
        ]

        # per-slot loads +16; per-stream-slot stores +16; compute sems +1
        s_in = [
            ctx.enter_context(nc.semaphore(f"s_in{i}")) for i in range(nslots)
        ]
        s_souts = [
            ctx.enter_context(nc.semaphore(f"s_souts{i}"))
            for i in range(max(nstream, 1))
        ]
        s_soutc = ctx.enter_context(nc.semaphore("s_soutc"))  # cached stores
        s_eye = ctx.enter_context(nc.semaphore("s_eye"))
        s_red = ctx.enter_context(nc.semaphore("s_red"))
        s_cmb = [
            ctx.enter_context(nc.semaphore(f"s_cmb{g}")) for g in range(ng)
        ]
        s_sqrt = [
            ctx.enter_context(nc.semaphore(f"s_sqrt{g}")) for g in range(ng)
        ]
        s_rcp = ctx.enter_context(nc.semaphore("s_rcp"))
        s_tp = [
            ctx.enter_context(nc.semaphore(f"s_tp{g}")) for g in range(ng)
        ]
        s_ptc = [
            ctx.enter_context(nc.semaphore(f"s_ptc{g}")) for g in range(ng)
        ]
        s_ccin = ctx.enter_context(nc.semaphore("s_ccin"))
        s_cc = ctx.enter_context(nc.semaphore("s_cc"))
        NCS = 2 * H  # column-scale broadcast chunks (quarters)
        w2 = n // NCS
        s_cs = [
            ctx.enter_context(nc.semaphore(f"s_cs{q}")) for q in range(NCS)
        ]
        s_stt = ctx.enter_context(nc.semaphore("s_stt"))
        block = ctx.enter_context(nc.Block())

        def item_src(i):
            t, h = items[i]
            return mx_v[t, :, h]

        def item_dst(i):
            t, h = items[i]
            return out_v[t, :, h]

        @block.gpsimd
        def _(g):
            # pass 1 loads
            for i in range(ni):
                if i in in_val2 and i >= nstream:
                    g.wait_ge(s_red, i - nstream + 1)  # slot's rowsum done
                g.dma_start(slots[slot_of(i)][:, :], item_src(i)).then_inc(
                    s_in[slot_of(i)], 16
                )

            # prefetch the first pass-2 stream loads (fills the AG window)
            if stream_items:
                g.wait_ge(s_red, len(stream_items))  # stream slots all free
            for i in stream_items[:nstream]:
                g.dma_start(slots[slot_of(i)][:, :], item_src(i)).then_inc(
                    s_in[slot_of(i)], 16
                )

            g.wait_ge(s_ccin, 16 * ng)  # SP wrote local r_inv to DRAM
            g.collective_compute(
                "AllGather",
                mybir.AluOpType.bypass,
                replica_groups=[list(range(ncores))],
                ins=[cc_in[:]],
                outs=[cc_out[:]],
            ).then_inc(s_cc, 1)

            # column-scale broadcast chunks: issued here (same engine as the
            # allgather -> no cross-engine hop) and on the Pool ring so the
            # stores on the SP ring are not queued behind 4 MiB of broadcast
            g.wait_ge(s_cc, 1)
            for q in range(NCS):
                g.dma_start(
                    colscale[:, q * w2 : (q + 1) * w2],
                    cc_out[q * w2 : (q + 1) * w2].partition_broadcast(P),
                ).then_inc(s_cs[q], 16)

            # remaining pass-2 stream loads (slot free when its store landed)
            for i in stream_items[nstream:]:
                g.wait_ge(s_souts[slot_of(i)], souts_val[i] - 16)
                g.dma_start(slots[slot_of(i)][:, :], item_src(i)).then_inc(
                    s_in[slot_of(i)], 16
                )

        @block.sync
        def _(sp):
            # identity for the PE transpose
            sp.dma_start(eye_sb[:, :], eye[:, :]).then_inc(s_eye, 16)
            # local r_inv (transposed via PE, staged to SBUF) -> DRAM
            for g, (a, b) in enumerate(groups):
                sp.wait_ge(s_ptc[g], 1)
                sp.dma_start(
                    cc_in[a * P : b * P], ptc[g][:, :]
                ).then_inc(s_ccin, 16)
            # stores, in pass-2 processing order
            for k, i in enumerate(order):
                sp.wait_ge(s_stt, k + 1)
                if i in in_val2:  # streamed
                    if souts_val[i] > 16:
                        sp.wait_ge(s_souts[slot_of(i)], souts_val[i] - 16)
                    sem = s_souts[slot_of(i)]
                else:
                    sem = s_soutc
                sp.dma_start(item_dst(i), slots[slot_of(i)][:, :]).then_inc(
                    sem, 16
                )
            # all stores landed before halt
            for s_idx in range(nstream):
                sp.wait_ge(s_souts[s_idx], souts_count[s_idx])
            if ncache:
                sp.wait_ge(s_soutc, 16 * ncache)

        @block.scalar
        def _(s):
            # pass 1: rowsums via in-place Copy with free-axis accumulate.
            # Group sqrts (in place on rs) are interleaved: group g's sqrt is
            # emitted right after the copies it depends on, so early groups'
            # sqrt runs in the gaps while later copies wait on their loads.
            done = 0
            for g, (a, b) in enumerate(groups):
                for i in range(done, b * H):
                    s.wait_ge(s_in[slot_of(i)], in_val1[i])
                    s.activation(
                        slots[slot_of(i)][:, :],
                        slots[slot_of(i)][:, :],
                        mybir.ActivationFunctionType.Copy,
                        accum_out=ps[:, i : i + 1],
                    ).then_inc(s_red, 1)
                done = b * H
                if b - a == 1:
                    # single-tile group: fuse half-combine + sqrt in one ACT
                    # op (no DVE round trip): sqrt(ps_even + ps_odd)
                    # (self-wait drains this engine's accum writebacks)
                    s.wait_ge(s_red, b * H)
                    s.activation(
                        rs[:, a:b],
                        ps[:, 2 * a : 2 * a + 1],
                        mybir.ActivationFunctionType.Sqrt,
                        bias=ps[:, 2 * a + 1 : 2 * a + 2],
                        scale=1.0,
                    ).then_inc(s_sqrt[g], 1)
                else:
                    s.wait_ge(s_cmb[g], 1)
                    s.sqrt(rs[:, a:b], rs[:, a:b]).then_inc(s_sqrt[g], 1)

        @block.tensor
        def _(pe):
            # sqrt(rowsum) [128, g] -> [g, 128] in PSUM (via identity)
            pe.wait_ge(s_eye, 16)
            for g, (a, b) in enumerate(groups):
                pe.wait_ge(s_sqrt[g], 1)
                pe.transpose(
                    pt[g][:, :], rs[:, a:b], eye_sb[:, :]
                ).then_inc(s_tp[g], 1)

        @block.vector
        def _(v):
            assert H == 2
            for g, (a, b) in enumerate(groups):
                if b - a > 1:
                    # combine halves: rs[:, t] = sum_h ps[:, t*H + h]
                    # (single-tile groups are fused into the ACT sqrt)
                    v.wait_ge(s_red, b * H)
                    v.scalar_tensor_tensor(
                        rs[:, a:b],
                        ps[:, 2 * a : 2 * b : 2],
                        1.0,
                        ps[:, 2 * a + 1 : 2 * b : 2],
                        op0=mybir.AluOpType.mult,
                        op1=mybir.AluOpType.add,
                    ).then_inc(s_cmb[g], 1)
                # row-scalar r_inv for the pass-2 scales
                v.wait_ge(s_sqrt[g], 1)
                v.reciprocal(rinv[:, a:b], rs[:, a:b]).then_inc(s_rcp, 1)
                # r_inv (transposed) = 1/transpose(sqrt): one fused step out
                # of PSUM, ready for the DRAM write
                v.wait_ge(s_tp[g], 1)
                v.reciprocal(ptc[g][:, :], pt[g][:, :]).then_inc(s_ptc[g], 1)
            # pass 2: fused row+column scale, in place
            # (self-wait drains the reciprocal writebacks before stts)
            v.wait_ge(s_rcp, ng)
            cs_seen = set()
            for i in order:
                t, h = items[i]
                for q in (2 * h, 2 * h + 1):
                    if q not in cs_seen:
                        cs_seen.add(q)
                        v.wait_ge(s_cs[q], 16)
                if i in in_val2:  # streamed: wait for its pass-2 load
                    v.wait_ge(s_in[slot_of(i)], in_val2[i])
                v.scalar_tensor_tensor(
                    slots[slot_of(i)][:, :],
                    slots[slot_of(i)][:, :],
                    rinv[:, t : t + 1],
                    colscale[:, h * w : (h + 1) * w],
                    op0=mybir.AluOpType.mult,
                    op1=mybir.AluOpType.mult,
                ).then_inc(s_stt, 1)

    return nc


_NC_CACHE = {}


def _get_nc(n=N, ncores=NCORES):
    key = (n, ncores)
    if key not in _NC_CACHE:
        _NC_CACHE[key] = build_kernel(n, ncores)
    return _NC_CACHE[key]


def kernel(adj, **run_kwargs):
    adj = np.asarray(adj)
    assert adj.shape == (N, N) and adj.dtype == np.float32
    mx = adj.copy()
    idx = np.arange(N)
    mx[idx, idx] += 1.0
    eye = np.eye(P, dtype=np.float32)

    in_maps = [
        {"mx": mx[c * SHARD : (c + 1) * SHARD], "eye": eye}
        for c in range(NCORES)
    ]
    nc = _get_nc()
    try:
        res = run_bass_kernel_spmd(nc, in_maps, list(range(NCORES)), **run_kwargs)
    except Exception:
        # transient device hiccups (e.g. a wedged core from an earlier
        # process) sometimes clear on a second attempt
        import time

        time.sleep(2.0)
        res = run_bass_kernel_spmd(nc, in_maps, list(range(NCORES)), **run_kwargs)
    out = np.concatenate([res.results[c]["out"] for c in range(NCORES)], axis=0)
    if run_kwargs:
        return out, res
    return out



# revision 13
# speedup vs baseline: 2.1202x; 2.1202x over previous
"""Normalized-adjacency kernel (EstimateAdj.normalize, symmetric=False) for TRN2.

out = mx * r_inv[:, None] * r_inv[None, :]   where mx = adj + I,
r_inv = rowsum(mx) ** -0.5.

Strategy (8 NeuronCores, row-sharded, raw Bass, fp16 data movement):
  - host: mx' = (adj + I) * 2^13 cast to fp16 (the scale keeps every nonzero
    element in fp16 normal range; the net 2^26 output scale is divided back
    out on the host, so no subnormal flush can hurt relative accuracy)
  - device, per core (shard = 1024 rows x 8192 cols fp16 = 16 MiB, fully
    resident in SBUF):
      pass 1: 8 tile loads [128 x 8192] split over two DMA rings (gpsimd +
              sync) so consecutive tiles stream in parallel; each tile's
              rowsum is split ACT (cols 0:4480, Copy+f32 accum) / DVE
              (cols 4480:, tensor_reduce) so reduces keep pace with loads
              and the post-load tail is ~4 us.
      r_inv' = 1/sqrt(rowsum * 2^-26): DVE add halves -> ACT sqrt (fp16) ->
      PE transpose -> DVE reciprocal -> DRAM; AllGather (fp16, 2 KiB/core).
      While the AllGather is in flight, DVE pre-applies the ROW scale
      (tensor_scalar_mul, local r_inv') to all 16 half-tiles in place.
      colscale: partition-broadcast the gathered row to [128 x 8192].
      pass 2: DVE tensor_tensor (tile *= colscale) per half, 16 stores of
              1 MiB on the sync ring.
  - host: upcast, divide by 2^26.

(remote_dma peer-write exchange would cut the collective cost to ~5 us but
InstRemoteDMA*/hostgen variants fail neuronxcc walrus codegen on this
toolchain: "ISA wrong length" in CoreV2GenImpl visitInstISA.)
"""

from contextlib import ExitStack

import numpy as np

import concourse.bass as bass
import concourse.mybir as mybir
from concourse.bass_utils import run_bass_kernel_spmd

N = 8192
NCORES = 8
SHARD = N // NCORES  # 1024
P = 128
T = SHARD // P  # 8 tiles per core
H = 2  # column halves per tile (store/TT granularity 4096)
CA = 4480  # ACT rowsum columns (rest go to DVE)

F16 = mybir.dt.float16
F32 = mybir.dt.float32

SCALE_IN = 8192.0  # 2^13
SCALE_OUT = float(2**26)


def build_kernel(n=N, ncores=NCORES, debug=False):
    shard = n // ncores
    tt = shard // P  # 8
    w = n // H  # 4096

    nc = bass.Bass(num_devices=ncores)
    mx = nc.dram_tensor("mx", [shard, n], F16, kind="ExternalInput")
    eye = nc.dram_tensor("eye", [P, P], F16, kind="ExternalInput")
    out = nc.dram_tensor("out", [shard, n], F16, kind="ExternalOutput")
    cc_in = nc.dram_tensor("cc_in", [shard], F16)
    if debug:
        o_psa = nc.dram_tensor("o_psa", [P, 8], F32, kind="ExternalOutput")
        o_psbs = nc.dram_tensor("o_psbs", [P, 8], F32, kind="ExternalOutput")
        o_rsqh = nc.dram_tensor("o_rsqh", [P, 8], F16, kind="ExternalOutput")
        o_ccin = nc.dram_tensor("o_ccin", [shard], F16, kind="ExternalOutput")
        o_ccout = nc.dram_tensor("o_ccout", [n], F16, kind="ExternalOutput")
        o_cs = nc.dram_tensor("o_cs", [P, n], F16, kind="ExternalOutput")
    cc_out = nc.dram_tensor("cc_out", [n], F16, addr_space="Shared")

    mx_l = mx.rearrange("(t p) m -> t p m", p=P)
    out_v = out.rearrange("(t p) (h w) -> t p h w", p=P, h=H)

    with ExitStack() as ctx:
        tiles = [
            ctx.enter_context(nc.sbuf_tensor(f"tile{t}", [P, n], F16))
            for t in range(tt)
        ]
        colscale = ctx.enter_context(nc.sbuf_tensor("colscale", [P, n], F16))
        eye_sb = ctx.enter_context(nc.sbuf_tensor("eye_sb", [P, P], F16))
        psa = ctx.enter_context(nc.sbuf_tensor("psa", [P, tt], F32))
        psb = ctx.enter_context(nc.sbuf_tensor("psb", [P, tt], F32))
        psbs = ctx.enter_context(nc.sbuf_tensor("psbs", [P, tt], F32))
        ps = ctx.enter_context(nc.sbuf_tensor("ps", [P, tt], F32))
        dr1 = ctx.enter_context(nc.sbuf_tensor("dr1", [P, 1], F32))
        dr2 = ctx.enter_context(nc.sbuf_tensor("dr2", [P, 1], F16))
        rsqh = ctx.enter_context(nc.sbuf_tensor("rsqh", [P, tt], F16))
        rx8 = ctx.enter_context(nc.sbuf_tensor("rx8", [P, tt], F32))
        ptc = ctx.enter_context(nc.sbuf_tensor("ptc", [tt, P], F16))
        pt = ctx.enter_context(nc.psum_tensor("pt", [tt, P], F16))

        s_in = [ctx.enter_context(nc.semaphore(f"s_in{t}")) for t in range(tt)]
        s_eye = ctx.enter_context(nc.semaphore("s_eye"))
        s_redA = ctx.enter_context(nc.semaphore("s_redA"))
        s_psb = ctx.enter_context(nc.semaphore("s_psb"))
        s_redAd = ctx.enter_context(nc.semaphore("s_redAd"))
        s_ps = ctx.enter_context(nc.semaphore("s_ps"))
        s_sqd = ctx.enter_context(nc.semaphore("s_sqd"))
        s_sq = ctx.enter_context(nc.semaphore("s_sq"))
        s_tpl = ctx.enter_context(nc.semaphore("s_tpl"))
        s_ptc = ctx.enter_context(nc.semaphore("s_ptc"))
        s_ccin = ctx.enter_context(nc.semaphore("s_ccin"))
        s_cc = ctx.enter_context(nc.semaphore("s_cc"))
        s_cs = [ctx.enter_context(nc.semaphore(f"s_cs{h}")) for h in range(H)]
        s_stt = ctx.enter_context(nc.semaphore("s_stt"))
        s_souts = ctx.enter_context(nc.semaphore("s_souts"))
        block = ctx.enter_context(nc.Block())

        @block.gpsimd
        def _(g):
            for t in range(tt):
                g.dma_start(tiles[t][:, :], mx_l[t, :, :]).then_inc(s_in[t], 16)
            g.wait_ge(s_ccin, 16)
            g.collective_compute(
                "AllGather",
                mybir.AluOpType.bypass,
                replica_groups=[list(range(ncores))],
                ins=[cc_in[:]],
                outs=[cc_out[:]],
            ).then_inc(s_cc, 1)
            # colscale broadcast, in halves so pass 2 starts on half 0
            g.wait_ge(s_cc, 1)
            for h in range(H):
                g.dma_start(
                    colscale[:, h * w : (h + 1) * w],
                    cc_out[h * w : (h + 1) * w].partition_broadcast(P),
                ).then_inc(s_cs[h], 16)

        @block.sync
        def _(sp):
            sp.dma_start(eye_sb[:, :], eye[:, :]).then_inc(s_eye, 16)
            # local r_inv' (transposed) -> DRAM for the AllGather
            sp.wait_ge(s_ptc, 1)
            sp.dma_start(cc_in[:], ptc[:, :]).then_inc(s_ccin, 16)
            if debug:
                sp.wait_ge(s_sqd, 1)
                sp.dma_start(o_psa[:, :], psa[:, :]).then_inc(s_souts, 16)
                sp.dma_start(o_psbs[:, :], psbs[:, :]).then_inc(s_souts, 16)
                sp.dma_start(o_rsqh[:, :], rsqh[:, :]).then_inc(s_souts, 16)
                sp.wait_ge(s_ccin, 16)
                sp.dma_start(o_ccin[:], cc_in[:]).then_inc(s_souts, 16)
                sp.wait_ge(s_cs[H - 1], 16)
                sp.dma_start(o_ccout[:], cc_out[:]).then_inc(s_souts, 16)
                sp.dma_start(o_cs[:, :], colscale[:, :]).then_inc(s_souts, 16)
                sp.wait_ge(s_souts, 96)
            # stores: tile-half k as soon as its col-scale lands
            k = 0
            extra = 96 if debug else 0
            for h in range(H):
                for t in range(tt):
                    k += 1
                    sp.wait_ge(s_stt, k)
                    sp.dma_start(
                        out_v[t, :, h], tiles[t][:, h * w : (h + 1) * w]
                    ).then_inc(s_souts, 16)
            sp.wait_ge(s_souts, 16 * tt * H + extra)

        @block.scalar
        def _(s):
            # rowsum half A per tile: in-place Copy with f32 accum
            for t in range(tt):
                s.wait_ge(s_in[t], 16)
                s.activation(
                    tiles[t][:, 0:CA],
                    tiles[t][:, 0:CA],
                    mybir.ActivationFunctionType.Copy,
                    accum_out=psa[:, t : t + 1],
                ).then_inc(s_redA, 1)
            # drain own accum writebacks (self-wait), then publish: the
            # dummy op's sem increment cannot fire before the drain, so a
            # cross-engine reader of psa gated on s_redAd is safe
            s.wait_ge(s_redA, tt)
            s.activation(
                dr1[:, :], psa[:, tt - 1 : tt],
                mybir.ActivationFunctionType.Copy,
            ).then_inc(s_redAd, 1)
            # rsq' = sqrt(rowsum * 2^-26)  (fp16 value ~0.7)
            s.wait_ge(s_ps, 1)
            s.activation(
                rsqh[:, :],
                ps[:, :],
                mybir.ActivationFunctionType.Sqrt,
                scale=1.0 / SCALE_OUT,
            ).then_inc(s_sq, 1)
            # drain + publish rsqh the same way for PE/DVE readers
            s.wait_ge(s_sq, 1)
            s.activation(
                dr2[:, :], rsqh[:, tt - 1 : tt],
                mybir.ActivationFunctionType.Copy,
            ).then_inc(s_sqd, 1)

        @block.tensor
        def _(pe):
            pe.wait_ge(s_eye, 16)
            pe.wait_ge(s_sqd, 1)
            pe.transpose(pt[:, :], rsqh[:, :], eye_sb[:, :]).then_inc(s_tpl, 1)

        @block.vector
        def _(v):
            # rowsum half B per tile
            for t in range(tt):
                v.wait_ge(s_in[t], 16)
                v.tensor_reduce(
                    psb[:, t : t + 1],
                    tiles[t][:, CA:n],
                    mybir.AxisListType.X,
                    mybir.AluOpType.add,
                )
            # combine rowsum halves (psa safe to read after s_redAd)
            v.wait_ge(s_redAd, 1)
            v.tensor_tensor(
                ps[:, :], psa[:, :], psb[:, :], mybir.AluOpType.add
            ).then_inc(s_ps, 1)
            # row scalars first (f32: tensor_scalar mult requires a float32
            # scalar operand); rsqh safe after the ACT drain-publish
            v.wait_ge(s_sqd, 1)
            v.reciprocal(rx8[:, :], rsqh[:, :])
            with nc.allow_low_precision(reason="fp16 r_inv, tol 2e-2"):
                # transposed reciprocal straight out of PSUM -> cc payload
                v.wait_ge(s_tpl, 1)
                v.reciprocal(ptc[:, :], pt[:, :]).then_inc(s_ptc, 1)
            # row scale, in place, while the AllGather is in flight
            for t in range(tt):
                for h in range(H):
                    v.tensor_scalar_mul(
                        tiles[t][:, h * w : (h + 1) * w],
                        tiles[t][:, h * w : (h + 1) * w],
                        rx8[:, t : t + 1],
                    )
            # pass 2: column scale, in place, half 0 first
            for h in range(H):
                v.wait_ge(s_cs[h], 16)
                for t in range(tt):
                    v.tensor_tensor(
                        tiles[t][:, h * w : (h + 1) * w],
                        tiles[t][:, h * w : (h + 1) * w],
                        colscale[:, h * w : (h + 1) * w],
                        mybir.AluOpType.mult,
                    ).then_inc(s_stt, 1)

    return nc


_NC_CACHE = {}


def _get_nc(n=N, ncores=NCORES):
    key = (n, ncores)
    if key not in _NC_CACHE:
        _NC_CACHE[key] = build_kernel(n, ncores)
    return _NC_CACHE[key]


def kernel(adj, **run_kwargs):
    adj = np.asarray(adj)
    assert adj.shape == (N, N) and adj.dtype == np.float32
    mxh = (adj * SCALE_IN).astype(np.float16)
    idx = np.arange(N)
    mxh[idx, idx] = (
        adj[idx, idx].astype(np.float64) * SCALE_IN + SCALE_IN
    ).astype(np.float16)
    eye = np.eye(P, dtype=np.float16)

    in_maps = [
        {"mx": mxh[c * SHARD : (c + 1) * SHARD], "eye": eye}
        for c in range(NCORES)
    ]
    nc = _get_nc()
    try:
        res = run_bass_kernel_spmd(nc, in_maps, list(range(NCORES)), **run_kwargs)
    except Exception:
        import time

        time.sleep(2.0)
        res = run_bass_kernel_spmd(nc, in_maps, list(range(NCORES)), **run_kwargs)

    full = np.concatenate(
        [res.results[c]["out"].astype(np.float32) for c in range(NCORES)],
        axis=0,
    ) / SCALE_OUT
    if run_kwargs:
        return full, res
    return full
